# revision 10
# baseline (speedup 1.0000x reference)
"""Data-parallel Trainium2 attention kernel (B=8 sharded over 8 NeuronCores).

Per core (one batch element):
  qkv projections (bf16 matmuls; q/k biases fused into the PSUM->SBUF copy
  via per-partition tensor_scalar add; softmax scale folded into Wq/bq on
  host; v/proj biases via K=1 ones-matmuls)
  scores computed transposed [k, q] via 16-way tile_position packing
  (4 heads x 4 k-chunks of 32, all concurrent in the 128x128 PE array)
  exp(s) on ACT for most kt-slots; tail slots use a Schraudolph bf16
  bit-trick on DVE (tensor_scalar f32->int16 viewed as bf16) so the
  group tail hides in freed ACT time
  bias applied as exp(s+rel) = exp(s)*exp(rel), host-precomputed exp(rel).T
  bf16; the multiply runs on DVE (most slots) or GPSIMD (kt 1,4)
  PV + softmax denominators via col-tiled matmuls (M=32 v-cols + M=1 ones)
  normalization: reciprocal_approx_fast on PSUM denominators + partition-
  broadcast DMA; the final multiplies run on GPSIMD to keep DVE flowing
  v/qk/proj chains are wedged between attention kt-slots to hide them
"""

import sys

sys.path.insert(0, "/opt/trn_rl_repo")

import numpy as np
import ml_dtypes

B, N, C, H, DH = 8, 1024, 768, 24, 32
HG = H // 4  # 6 groups of 4 heads
SCALE = DH ** -0.5
BF16 = ml_dtypes.bfloat16

SCH_A = 128.0 / np.log(2.0)          # Schraudolph: bf16 bits = s*A + C
SCH_C = 127.0 * 128.0 - 5.0 + 0.5

_CACHE = {}


def _build():
    if "nc" in _CACHE:
        return _CACHE["nc"]
    from contextlib import ExitStack
    import concourse.mybir as mybir
    import concourse.tile as tile
    from concourse import bacc

    nc = bacc.Bacc("TRN2")
    bf, f32 = mybir.dt.bfloat16, mybir.dt.float32
    i16 = mybir.dt.int16
    Exp = mybir.ActivationFunctionType.Exp
    Add = mybir.AluOpType.add
    Mult = mybir.AluOpType.mult

    xT_d = nc.declare_dram_parameter("xT", [128, 6, N], bf, isOutput=False)
    wq_d = nc.declare_dram_parameter("wq", [128, 6, C], bf, isOutput=False)
    wk_d = nc.declare_dram_parameter("wk", [128, 6, C], bf, isOutput=False)
    wv_d = nc.declare_dram_parameter("wv", [128, 6, C], bf, isOutput=False)
    bqt_d = nc.declare_dram_parameter("bqt", [128, 6], f32, isOutput=False)
    bkt_d = nc.declare_dram_parameter("bkt", [128, 6], f32, isOutput=False)
    eb_d = nc.declare_dram_parameter("expb", [HG, N, 4, N], bf, isOutput=False)
    wp_d = nc.declare_dram_parameter("wpj", [128, 6, C], bf, isOutput=False)
    bp_d = nc.declare_dram_parameter("bpj", [1, C], bf, isOutput=False)
    out_d = nc.declare_dram_parameter("out", [N, C], bf, isOutput=True)
    scr_d = nc.dram_tensor("recscr", [128, 12, 512], bf)

    with tile.TileContext(nc) as tc, ExitStack() as ctx:
        ctx.enter_context(nc.allow_low_precision(
            reason="bf16 compute intentional; rel_err budget 2e-2"))
        const = ctx.enter_context(tc.tile_pool(name="const", bufs=1))
        big = ctx.enter_context(tc.tile_pool(name="big", bufs=1))
        prb = ctx.enter_context(tc.tile_pool(name="prb", bufs=4))
        stage = ctx.enter_context(tc.tile_pool(name="stage", bufs=6))
        norm = ctx.enter_context(tc.tile_pool(name="norm", bufs=2))
        ypool = ctx.enter_context(tc.tile_pool(name="ypool", bufs=3))
        psA = ctx.enter_context(tc.tile_pool(name="psA", bufs=3, space="PSUM"))
        psB = ctx.enter_context(tc.tile_pool(name="psB", bufs=2, space="PSUM"))

        # ---- constant loads (wp deferred: not needed until proj)
        xT = const.tile([128, 6, N], bf)
        nc.sync.dma_start(out=xT, in_=xT_d[:, :, :])
        wq = const.tile([128, 6, C], bf)
        nc.sync.dma_start(out=wq, in_=wq_d[:, :, :])
        wk = const.tile([128, 6, C], bf)
        nc.sync.dma_start(out=wk, in_=wk_d[:, :, :])
        wv = const.tile([128, 6, C], bf)
        nc.sync.dma_start(out=wv, in_=wv_d[:, :, :])
        bqt = const.tile([128, 6], f32)
        nc.sync.dma_start(out=bqt, in_=bqt_d[:, :])
        bkt = const.tile([128, 6], f32)
        nc.sync.dma_start(out=bkt, in_=bkt_d[:, :])
        bp = const.tile([1, C], bf)
        nc.sync.dma_start(out=bp, in_=bp_d[:, :])
        ones512 = const.tile([1, 512], bf)
        nc.vector.memset(ones512, 1.0)
        ones128 = const.tile([128, 1], bf)
        nc.vector.memset(ones128, 1.0)
        wp = const.tile([128, 6, C], bf)

        # ---- persistent intermediates
        qT = big.tile([128, 6, N], bf)      # q*scale+bq, [32h+d -> (p,s)], n
        kT = big.tile([128, 6, N], bf)
        v = big.tile([128, 8, H, DH], bf)   # [token%128, token//128, h, d]
        outT = big.tile([128, 6, N], bf)    # unnorm attn out.T [32h+d, n]

        # ---- projection chain generators (one PSUM-tile chain per call)
        def qk_chain(j, which, t):
            wt, bt, dstT = ((wq, bqt, qT), (wk, bkt, kT))[which]
            ps = psA.tile([128, 2, 512], f32, tag="ps")
            pq = ps[:, 0, :]
            for s in range(6):
                nc.tensor.matmul(
                    pq,
                    lhsT=wt[:, s, 128 * j:128 * (j + 1)],
                    rhs=xT[:, s, 512 * t:512 * (t + 1)],
                    start=(s == 0), stop=(s == 5))
            nc.vector.tensor_scalar(
                out=dstT[:, j, 512 * t:512 * (t + 1)], in0=pq,
                scalar1=bt[:, j:j + 1], scalar2=None, op0=Add)

        def v_chain(i, half):
            f0, fw = ((0, 512), (512, 256))[half]
            ps = psA.tile([128, 2, 512], f32, tag="ps")
            pv_ = ps[:, 0, :fw]
            for s in range(6):
                nc.tensor.matmul(
                    pv_,
                    lhsT=xT[:, s, 128 * i:128 * (i + 1)],
                    rhs=wv[:, s, f0:f0 + fw],
                    start=(s == 0), stop=(s == 5))
            nc.vector.tensor_copy(
                out=v[:, i, f0 // DH:(f0 + fw) // DH, :],
                in_=pv_.rearrange("p (h d) -> p h d", d=DH))

        def proj_chain(i, half, ytile):
            f0, fw = ((0, 512), (512, 256))[half]
            ps = psA.tile([128, 2, 512], f32, tag="ps")
            py = ps[:, 0, :fw]
            for s in range(6):
                nc.tensor.matmul(
                    py,
                    lhsT=outT[:, s, 128 * i:128 * (i + 1)],
                    rhs=wp[:, s, f0:f0 + fw],
                    start=(s == 0), stop=False)
            nc.tensor.matmul(
                py, lhsT=ones512[:, :128], rhs=bp[:, f0:f0 + fw],
                start=False, stop=True)
            nc.vector.tensor_copy(out=ytile[:, f0:f0 + fw], in_=py)
            if half == 1:
                nc.sync.dma_start(
                    out=out_d[128 * i:128 * (i + 1), :], in_=ytile)

        # ---- attention, split in two phases so PV of group g can be
        # wedged into the scores slots of group g+1 (keeps ACT streaming)
        def emit_scores(g, qt, wedges):
            qs = slice(512 * qt, 512 * (qt + 1))
            # probs layout [p, kt, hl, q] split in two half-tiles (kt 0-3,
            # 4-7) so the pool frees at half-group granularity
            p_lo = prb.tile([128, 4, 4, 512], bf, tag="probs",
                            name=f"probsL{qt}{g}")
            p_hi = prb.tile([128, 4, 4, 512], bf, tag="probs",
                            name=f"probsH{qt}{g}")
            probs = (lambda lo, hi: lambda kt: (lo if kt < 4 else hi)[
                :, kt % 4])(p_lo, p_hi)
            wi = iter(wedges)
            nw = len(wedges)
            ndone = 0
            for kt in range(8):
                eb = stage.tile([128, 4, 512], bf, tag="eb")
                nc.sync.dma_start(
                    out=eb, in_=eb_d[g, 128 * kt:128 * (kt + 1), :, qs])
                scA = psA.tile([128, 2, 512], f32, tag="ps")
                scB = psA.tile([128, 2, 512], f32, tag="ps")
                # 16-way packing: head i -> row 32i, k-chunk j -> col 32j
                for i in range(4):
                    sc = scA if i < 2 else scB
                    for j in range(4):
                        nc.tensor.matmul(
                            sc[32 * j:32 * (j + 1), i % 2, :],
                            lhsT=kT[32 * i:32 * (i + 1), g,
                                    128 * kt + 32 * j:128 * kt + 32 * (j + 1)],
                            rhs=qT[32 * i:32 * (i + 1), g, qs],
                            start=True, stop=True,
                            tile_position=(32 * i, 32 * j),
                            skip_group_check=True)
                pk = probs(kt)
                nc.scalar.activation(out=pk[:, 0:2, :], in_=scA, func=Exp)
                nc.scalar.activation(out=pk[:, 2:4, :], in_=scB, func=Exp)
                nc.vector.tensor_tensor(
                    out=pk[:, :, :], in0=pk[:, :, :], in1=eb, op=Mult)
                while ndone * 8 < nw * (kt + 1):
                    w = next(wi, None)
                    ndone += 1
                    if w is not None:
                        w()
            for w in wi:
                w()
            return probs

        def emit_pv(g, qt, probs):
            """Returns wedge list: 8 kt-quads + 1 finisher."""
            qs = slice(512 * qt, 512 * (qt + 1))
            st = {}

            def quad(kt):
                if kt == 0:
                    st["pv"] = psB.tile([128, 512], f32, tag="pv",
                                        name=f"pv{qt}{g}")
                    st["dn"] = psB.tile([128, 512], f32, tag="pv",
                                        name=f"dn{qt}{g}")
                    nc.vector.memset(st["dn"], 1.0)
                pv, dn = st["pv"], st["dn"]
                for hl in range(4):
                    nc.tensor.matmul(
                        pv[32 * hl:32 * (hl + 1), :],
                        lhsT=v[:, kt, 4 * g + hl, :],
                        rhs=probs(kt)[:, hl, :],
                        start=(kt == 0), stop=(kt == 7),
                        tile_position=(0, 32 * hl),
                        skip_group_check=True)
                    nc.tensor.matmul(
                        dn[32 * hl:32 * hl + 1, :],
                        lhsT=ones128[:, :],
                        rhs=probs(kt)[:, hl, :],
                        start=(kt == 0), stop=(kt == 7),
                        tile_position=(0, 32 * hl),
                        skip_group_check=True)

            def finish():
                pv, dn = st["pv"], st["dn"]
                nc.vector.tensor_copy(out=outT[:, g, qs], in_=pv)
                slot = 6 * qt + g
                rec = norm.tile([128, 512], f32, tag="rec")
                dtile = norm.tile([128, 512], bf, tag="den")
                rtile = norm.tile([128, 512], bf, tag="rb")
                nc.vector.reciprocal_approx_fast(out=rec, in_=dn)
                nc.gpsimd.tensor_copy(out=dtile, in_=rec)
                nc.sync.dma_start(out=scr_d[:, slot, :], in_=dtile)
                for hl in range(4):
                    nc.sync.dma_start(
                        out=rtile[32 * hl:32 * (hl + 1), :],
                        in_=scr_d[32 * hl:32 * hl + 1, slot, :].to_broadcast(
                            (32, 512)))
                nc.gpsimd.tensor_tensor(
                    out=outT[:, g, qs], in0=outT[:, g, qs], in1=rtile,
                    op=Mult)

            return [lambda kt=kt: quad(kt) for kt in range(8)] + [finish]

        # ---- schedule: qk(0) first so attention(0,0) starts ~6us in; v and
        # the remaining qk groups ride inside attention kt-slots; proj per qt
        def wedge(fn, *a):
            return lambda: fn(*a)

        def merge(a, b):
            out = []
            ia = ib = 0
            while ia < len(a) or ib < len(b):
                if ib >= len(b) or (ia < len(a) and ia * len(b) <= ib * len(a)):
                    out.append(a[ia]); ia += 1
                else:
                    out.append(b[ib]); ib += 1
            return out

        # head: only the chains the first scores group needs (q t0, k both)
        qk_chain(0, 0, 0)
        qk_chain(0, 1, 0)
        qk_chain(0, 1, 1)
        v_w = [wedge(v_chain, i, h) for i in range(8) for h in range(2)]
        qk_w = {j: [wedge(qk_chain, j, w, t) for w in range(2)
                    for t in range(2)] for j in range(1, 6)}
        # wedges for group g carry PV of the previous group, plus qk(g+1)
        # one group ahead / proj chunks in qt=1
        p = emit_scores(0, 0, merge([wedge(qk_chain, 0, 0, 1)] + qk_w[1], v_w))
        pw = emit_pv(0, 0, p)
        p = emit_scores(1, 0, merge(pw, qk_w[2]))
        pw = emit_pv(1, 0, p)
        p = emit_scores(2, 0, merge(pw, qk_w[3]))
        pw = emit_pv(2, 0, p)
        p = emit_scores(3, 0, merge(pw, qk_w[4]))
        pw = emit_pv(3, 0, p)
        p = emit_scores(4, 0, merge(pw, qk_w[5]))
        pw = emit_pv(4, 0, p)
        p = emit_scores(5, 0, pw)
        pw = emit_pv(5, 0, p)
        nc.sync.dma_start(out=wp, in_=wp_d[:, :, :])
        p = emit_scores(0, 1, pw)
        pw = emit_pv(0, 1, p)
        yts = [ypool.tile([128, C], bf, tag="y", name=f"yt{i}")
               for i in range(8)]
        p = emit_scores(1, 1, merge(pw, [wedge(proj_chain, 0, 0, yts[0]),
                                         wedge(proj_chain, 0, 1, yts[0])]))
        pw = emit_pv(1, 1, p)
        p = emit_scores(2, 1, merge(pw, [wedge(proj_chain, 1, 0, yts[1]),
                                         wedge(proj_chain, 1, 1, yts[1])]))
        pw = emit_pv(2, 1, p)
        p = emit_scores(3, 1, merge(pw, [wedge(proj_chain, 2, 0, yts[2]),
                                         wedge(proj_chain, 2, 1, yts[2])]))
        pw = emit_pv(3, 1, p)
        p = emit_scores(4, 1, merge(pw, [wedge(proj_chain, 3, 0, yts[3]),
                                         wedge(proj_chain, 3, 1, yts[3])]))
        pw = emit_pv(4, 1, p)
        p = emit_scores(5, 1, pw)
        for w in emit_pv(5, 1, p):
            w()
        for i in range(4, 8):
            proj_chain(i, 0, yts[i])
            proj_chain(i, 1, yts[i])

    nc.finalize()
    _CACHE["nc"] = nc
    return nc


def _prep_shared(shared_rel_pos, Wqkv, bqkv, Wproj, bproj):
    """Host-side weight rearrangement shared by all cores (float32 in)."""
    w3 = np.asarray(Wqkv, np.float32).reshape(H, 3, DH, C)
    wq_t = (w3[:, 0] * SCALE).transpose(2, 0, 1).reshape(C, C)
    wk_t = w3[:, 1].transpose(2, 0, 1).reshape(C, C)
    wv_t = w3[:, 2].transpose(2, 0, 1).reshape(C, C)
    b3 = np.asarray(bqkv, np.float32).reshape(H, 3, DH)
    bq_a = (b3[:, 0] * SCALE).reshape(C)
    bk_a = b3[:, 1].reshape(C)
    bv_a = b3[:, 2].reshape(1, C)
    # exp(rel)^T grouped: [g, k, hl, q]
    expb = np.exp(np.asarray(shared_rel_pos, np.float32))
    expb = expb.transpose(0, 2, 1).reshape(HG, 4, N, N).transpose(0, 2, 1, 3)
    wp_t = np.asarray(Wproj, np.float32).T.copy()
    bp_a = (np.asarray(bproj, np.float32) +
            np.asarray(Wproj, np.float32) @ bv_a.reshape(C)).reshape(1, C)
    def p_major(w):  # [C_out-as-(s p), m] -> [p, s, m]
        return np.ascontiguousarray(
            w.reshape(6, 128, -1).transpose(1, 0, 2))
    return {
        "wq": p_major(wq_t).astype(BF16),
        "wk": p_major(wk_t).astype(BF16),
        "wv": p_major(wv_t).astype(BF16),
        "bqt": np.ascontiguousarray(bq_a.reshape(6, 128).T),
        "bkt": np.ascontiguousarray(bk_a.reshape(6, 128).T),
        "expb": np.ascontiguousarray(expb).astype(BF16),
        "wpj": p_major(wp_t).astype(BF16),
        "bpj": bp_a.astype(BF16),
    }


def _in_maps(x, shared):
    x = np.asarray(x, np.float32)
    maps = []
    for b in range(B):
        m = dict(shared)
        m["xT"] = np.ascontiguousarray(
            x[b].T.reshape(6, 128, N).transpose(1, 0, 2)).astype(BF16)
        maps.append(m)
    return maps


def kernel(**inputs):
    from concourse.bass_utils import run_bass_kernel_spmd

    nc = _build()
    shared = _prep_shared(
        inputs["shared_rel_pos"], inputs["Wqkv"], inputs["bqkv"],
        inputs["Wproj"], inputs["bproj"])
    maps = _in_maps(inputs["x"], shared)
    res = run_bass_kernel_spmd(nc, maps, core_ids=list(range(B)))
    out = np.stack([np.asarray(res.results[i]["out"], np.float32)
                    for i in range(B)])
    return out


# revision 11
# speedup vs baseline: 1.2147x; 1.2147x over previous
"""Data-parallel Trainium2 attention kernel (B=8 sharded over 8 NeuronCores).

Per core (one batch element):
  qkv projections (bf16 matmuls; q/k biases fused into the PSUM->SBUF copy
  via per-partition tensor_scalar add; softmax scale folded into Wq/bq on
  host; v/proj biases via K=1 ones-matmuls)
  scores computed transposed [k, q] via 16-way tile_position packing
  (4 heads x 4 k-chunks of 32, all concurrent in the 128x128 PE array)
  exp(s) on ACT for most kt-slots; tail slots use a Schraudolph bf16
  bit-trick on DVE (tensor_scalar f32->int16 viewed as bf16) so the
  group tail hides in freed ACT time
  bias applied as exp(s+rel) = exp(s)*exp(rel), host-precomputed exp(rel).T
  bf16; the multiply runs on DVE (most slots) or GPSIMD (kt 1,4)
  PV + softmax denominators via col-tiled matmuls (M=32 v-cols + M=1 ones)
  normalization: reciprocal_approx_fast on PSUM denominators + partition-
  broadcast DMA; the final multiplies run on GPSIMD to keep DVE flowing
  v/qk/proj chains are wedged between attention kt-slots to hide them
"""

import sys

sys.path.insert(0, "/opt/trn_rl_repo")

import numpy as np
import ml_dtypes

B, N, C, H, DH = 8, 1024, 768, 24, 32
HG = H // 4  # 6 groups of 4 heads
SCALE = DH ** -0.5
BF16 = ml_dtypes.bfloat16

SCH_A = 128.0 / np.log(2.0)          # Schraudolph: bf16 bits = s*A + C
SCH_C = 127.0 * 128.0 - 5.0 + 0.5

_CACHE = {}


def _build():
    if "nc" in _CACHE:
        return _CACHE["nc"]
    from contextlib import ExitStack
    import concourse.mybir as mybir
    import concourse.tile as tile
    from concourse import bacc

    nc = bacc.Bacc("TRN2")
    bf, f32 = mybir.dt.bfloat16, mybir.dt.float32
    i16 = mybir.dt.int16
    Exp = mybir.ActivationFunctionType.Exp
    Add = mybir.AluOpType.add
    Mult = mybir.AluOpType.mult

    xT_d = nc.declare_dram_parameter("xT", [128, 6, N], bf, isOutput=False)
    wq_d = nc.declare_dram_parameter("wq", [128, 6, C], bf, isOutput=False)
    wk_d = nc.declare_dram_parameter("wk", [128, 6, C], bf, isOutput=False)
    wv_d = nc.declare_dram_parameter("wv", [128, 6, C], bf, isOutput=False)
    bqt_d = nc.declare_dram_parameter("bqt", [128, 6], f32, isOutput=False)
    bkt_d = nc.declare_dram_parameter("bkt", [128, 6], f32, isOutput=False)
    eb_d = nc.declare_dram_parameter("expb", [HG, N, 4, N], bf, isOutput=False)
    wp_d = nc.declare_dram_parameter("wpj", [128, 6, C], bf, isOutput=False)
    bp_d = nc.declare_dram_parameter("bpj", [1, C], bf, isOutput=False)
    out_d = nc.declare_dram_parameter("out", [N, C], bf, isOutput=True)
    scr_d = nc.dram_tensor("recscr", [128, 12, 512], bf)

    with tile.TileContext(nc) as tc, ExitStack() as ctx:
        ctx.enter_context(nc.allow_low_precision(
            reason="bf16 compute intentional; rel_err budget 2e-2"))
        const = ctx.enter_context(tc.tile_pool(name="const", bufs=1))
        big = ctx.enter_context(tc.tile_pool(name="big", bufs=1))
        prb = ctx.enter_context(tc.tile_pool(name="prb", bufs=2))
        stage = ctx.enter_context(tc.tile_pool(name="stage", bufs=5))
        norm = ctx.enter_context(tc.tile_pool(name="norm", bufs=2))
        ypool = ctx.enter_context(tc.tile_pool(name="ypool", bufs=3))
        psA = ctx.enter_context(tc.tile_pool(name="psA", bufs=3, space="PSUM"))
        psB = ctx.enter_context(tc.tile_pool(name="psB", bufs=2, space="PSUM"))

        # ---- constant loads (wp deferred: not needed until proj)
        xT = const.tile([128, 6, N], bf)
        nc.sync.dma_start(out=xT, in_=xT_d[:, :, :])
        wq = const.tile([128, 6, C], bf)
        nc.sync.dma_start(out=wq, in_=wq_d[:, :, :])
        wk = const.tile([128, 6, C], bf)
        nc.sync.dma_start(out=wk, in_=wk_d[:, :, :])
        wv = const.tile([128, 6, C], bf)
        nc.sync.dma_start(out=wv, in_=wv_d[:, :, :])
        bqt = const.tile([128, 6], f32)
        nc.sync.dma_start(out=bqt, in_=bqt_d[:, :])
        bkt = const.tile([128, 6], f32)
        nc.sync.dma_start(out=bkt, in_=bkt_d[:, :])
        bp = const.tile([1, C], bf)
        nc.sync.dma_start(out=bp, in_=bp_d[:, :])
        ones512 = const.tile([1, 512], bf)
        nc.vector.memset(ones512, 1.0)
        ones128 = const.tile([128, 1], bf)
        nc.vector.memset(ones128, 1.0)
        wp = const.tile([128, 6, C], bf)

        # ---- persistent intermediates
        qT = big.tile([128, 6, N], bf)      # q*scale+bq, [32h+d -> (p,s)], n
        kT = big.tile([128, 6, N], bf)
        v = big.tile([128, 8, H, DH], bf)   # [token%128, token//128, h, d]
        outT = big.tile([128, 6, N], bf)    # unnorm attn out.T [32h+d, n]

        # ---- projection chain generators (one PSUM-tile chain per call)
        def qk_chain(j, which, t):
            wt, bt, dstT = ((wq, bqt, qT), (wk, bkt, kT))[which]
            ps = psA.tile([128, 2, 512], f32, tag="ps")
            pq = ps[:, 0, :]
            for s in range(6):
                nc.tensor.matmul(
                    pq,
                    lhsT=wt[:, s, 128 * j:128 * (j + 1)],
                    rhs=xT[:, s, 512 * t:512 * (t + 1)],
                    start=(s == 0), stop=(s == 5))
            nc.vector.tensor_scalar(
                out=dstT[:, j, 512 * t:512 * (t + 1)], in0=pq,
                scalar1=bt[:, j:j + 1], scalar2=None, op0=Add)

        def v_chain(i, half):
            f0, fw = ((0, 512), (512, 256))[half]
            ps = psA.tile([128, 2, 512], f32, tag="ps")
            pv_ = ps[:, 0, :fw]
            for s in range(6):
                nc.tensor.matmul(
                    pv_,
                    lhsT=xT[:, s, 128 * i:128 * (i + 1)],
                    rhs=wv[:, s, f0:f0 + fw],
                    start=(s == 0), stop=(s == 5))
            nc.vector.tensor_copy(
                out=v[:, i, f0 // DH:(f0 + fw) // DH, :],
                in_=pv_.rearrange("p (h d) -> p h d", d=DH))

        def proj_chain(i, half, ytile):
            f0, fw = ((0, 512), (512, 256))[half]
            ps = psA.tile([128, 2, 512], f32, tag="ps")
            py = ps[:, 0, :fw]
            for s in range(6):
                nc.tensor.matmul(
                    py,
                    lhsT=outT[:, s, 128 * i:128 * (i + 1)],
                    rhs=wp[:, s, f0:f0 + fw],
                    start=(s == 0), stop=False)
            nc.tensor.matmul(
                py, lhsT=ones512[:, :128], rhs=bp[:, f0:f0 + fw],
                start=False, stop=True)
            nc.vector.tensor_copy(out=ytile[:, f0:f0 + fw], in_=py)
            if half == 1:
                nc.sync.dma_start(
                    out=out_d[128 * i:128 * (i + 1), :], in_=ytile)

        # ---- attention, split in two phases so PV of group g can be
        # wedged into the scores slots of group g+1 (keeps ACT streaming)
        def emit_scores(g, qt, wedges):
            qs = slice(512 * qt, 512 * (qt + 1))
            # probs layout [p, kt, hl, q]: each kt slice is N=2048-contig
            pr = prb.tile([128, 8, 4, 512], bf, tag="probs",
                          name=f"probs{qt}{g}")
            probs = (lambda t: lambda kt: t[:, kt])(pr)
            wi = iter(wedges)
            nw = len(wedges)
            for kt in range(8):
                eb = stage.tile([128, 4, 512], bf, tag="eb")
                nc.sync.dma_start(
                    out=eb, in_=eb_d[g, 128 * kt:128 * (kt + 1), :, qs])
                scA = psA.tile([128, 2, 512], f32, tag="ps")
                scB = psA.tile([128, 2, 512], f32, tag="ps")
                # 16-way packing: head i -> row 32i, k-chunk j -> col 32j
                for i in range(4):
                    sc = scA if i < 2 else scB
                    for j in range(4):
                        nc.tensor.matmul(
                            sc[32 * j:32 * (j + 1), i % 2, :],
                            lhsT=kT[32 * i:32 * (i + 1), g,
                                    128 * kt + 32 * j:128 * kt + 32 * (j + 1)],
                            rhs=qT[32 * i:32 * (i + 1), g, qs],
                            start=True, stop=True,
                            tile_position=(32 * i, 32 * j),
                            skip_group_check=True)
                pk = probs(kt)
                nc.scalar.activation(out=pk[:, 0:2, :], in_=scA, func=Exp)
                nc.scalar.activation(out=pk[:, 2:4, :], in_=scB, func=Exp)
                nc.vector.tensor_tensor(
                    out=pk[:, :, :], in0=pk[:, :, :], in1=eb, op=Mult)
                for _ in range(2 if nw > 8 else 1):
                    w = next(wi, None)
                    if w is not None:
                        w()
            for w in wi:
                w()
            return probs

        def emit_pv(g, qt, probs):
            """Returns wedge list: 8 kt-quads + 1 finisher."""
            qs = slice(512 * qt, 512 * (qt + 1))
            st = {}

            def quad(kt):
                if kt == 0:
                    st["pv"] = psB.tile([128, 512], f32, tag="pv",
                                        name=f"pv{qt}{g}")
                    st["dn"] = psB.tile([128, 512], f32, tag="pv",
                                        name=f"dn{qt}{g}")
                    nc.vector.memset(st["dn"], 1.0)
                pv, dn = st["pv"], st["dn"]
                for hl in range(4):
                    nc.tensor.matmul(
                        pv[32 * hl:32 * (hl + 1), :],
                        lhsT=v[:, kt, 4 * g + hl, :],
                        rhs=probs(kt)[:, hl, :],
                        start=(kt == 0), stop=(kt == 7),
                        tile_position=(0, 32 * hl),
                        skip_group_check=True)
                    nc.tensor.matmul(
                        dn[32 * hl:32 * hl + 1, :],
                        lhsT=ones128[:, :],
                        rhs=probs(kt)[:, hl, :],
                        start=(kt == 0), stop=(kt == 7),
                        tile_position=(0, 32 * hl),
                        skip_group_check=True)

            def finish():
                pv, dn = st["pv"], st["dn"]
                nc.vector.tensor_copy(out=outT[:, g, qs], in_=pv)
                slot = 6 * qt + g
                rec = norm.tile([128, 512], f32, tag="rec")
                dtile = norm.tile([128, 512], bf, tag="den")
                rtile = norm.tile([128, 512], bf, tag="rb")
                nc.vector.reciprocal_approx_fast(out=rec, in_=dn)
                nc.gpsimd.tensor_copy(out=dtile, in_=rec)
                nc.sync.dma_start(out=scr_d[:, slot, :], in_=dtile)
                for hl in range(4):
                    nc.sync.dma_start(
                        out=rtile[32 * hl:32 * (hl + 1), :],
                        in_=scr_d[32 * hl:32 * hl + 1, slot, :].to_broadcast(
                            (32, 512)))
                nc.gpsimd.tensor_tensor(
                    out=outT[:, g, qs], in0=outT[:, g, qs], in1=rtile,
                    op=Mult)

            return [lambda kt=kt: quad(kt) for kt in range(8)] + [finish]

        # ---- schedule: qk(0) first so attention(0,0) starts ~6us in; v and
        # the remaining qk groups ride inside attention kt-slots; proj per qt
        def wedge(fn, *a):
            return lambda: fn(*a)

        def merge(a, b):
            out = []
            ia = ib = 0
            while ia < len(a) or ib < len(b):
                if ib >= len(b) or (ia < len(a) and ia * len(b) <= ib * len(a)):
                    out.append(a[ia]); ia += 1
                else:
                    out.append(b[ib]); ib += 1
            return out

        # head: only the chains the first scores group needs (q t0, k both)
        qk_chain(0, 0, 0)
        qk_chain(0, 1, 0)
        qk_chain(0, 1, 1)
        v_w = [wedge(v_chain, i, h) for i in range(8) for h in range(2)]
        qk_w = {j: [wedge(qk_chain, j, w, t) for w in range(2)
                    for t in range(2)] for j in range(1, 6)}
        # wedges for group g carry PV of the previous group, plus qk(g+1)
        # one group ahead / proj chunks in qt=1
        p = emit_scores(0, 0, [wedge(qk_chain, 0, 0, 1)] + qk_w[1] + v_w)
        pw = emit_pv(0, 0, p)
        p = emit_scores(1, 0, pw + qk_w[2])
        pw = emit_pv(1, 0, p)
        p = emit_scores(2, 0, pw + qk_w[3])
        pw = emit_pv(2, 0, p)
        p = emit_scores(3, 0, pw + qk_w[4])
        pw = emit_pv(3, 0, p)
        p = emit_scores(4, 0, pw + qk_w[5])
        pw = emit_pv(4, 0, p)
        p = emit_scores(5, 0, pw)
        pw = emit_pv(5, 0, p)
        nc.sync.dma_start(out=wp, in_=wp_d[:, :, :])
        p = emit_scores(0, 1, pw)
        pw = emit_pv(0, 1, p)
        yts = [ypool.tile([128, C], bf, tag="y", name=f"yt{i}")
               for i in range(8)]
        p = emit_scores(1, 1, pw + [wedge(proj_chain, 0, 0, yts[0]),
                                    wedge(proj_chain, 0, 1, yts[0])])
        pw = emit_pv(1, 1, p)
        p = emit_scores(2, 1, pw + [wedge(proj_chain, 1, 0, yts[1]),
                                    wedge(proj_chain, 1, 1, yts[1])])
        pw = emit_pv(2, 1, p)
        p = emit_scores(3, 1, pw + [wedge(proj_chain, 2, 0, yts[2]),
                                    wedge(proj_chain, 2, 1, yts[2])])
        pw = emit_pv(3, 1, p)
        p = emit_scores(4, 1, pw + [wedge(proj_chain, 3, 0, yts[3]),
                                    wedge(proj_chain, 3, 1, yts[3])])
        pw = emit_pv(4, 1, p)
        p = emit_scores(5, 1, pw)
        for w in emit_pv(5, 1, p):
            w()
        for i in range(4, 8):
            proj_chain(i, 0, yts[i])
            proj_chain(i, 1, yts[i])

    nc.finalize()
    _CACHE["nc"] = nc
    return nc


def _prep_shared(shared_rel_pos, Wqkv, bqkv, Wproj, bproj):
    """Host-side weight rearrangement shared by all cores (float32 in)."""
    w3 = np.asarray(Wqkv, np.float32).reshape(H, 3, DH, C)
    wq_t = (w3[:, 0] * SCALE).transpose(2, 0, 1).reshape(C, C)
    wk_t = w3[:, 1].transpose(2, 0, 1).reshape(C, C)
    wv_t = w3[:, 2].transpose(2, 0, 1).reshape(C, C)
    b3 = np.asarray(bqkv, np.float32).reshape(H, 3, DH)
    bq_a = (b3[:, 0] * SCALE).reshape(C)
    bk_a = b3[:, 1].reshape(C)
    bv_a = b3[:, 2].reshape(1, C)
    # exp(rel)^T grouped: [g, k, hl, q]
    expb = np.exp(np.asarray(shared_rel_pos, np.float32))
    expb = expb.transpose(0, 2, 1).reshape(HG, 4, N, N).transpose(0, 2, 1, 3)
    wp_t = np.asarray(Wproj, np.float32).T.copy()
    bp_a = (np.asarray(bproj, np.float32) +
            np.asarray(Wproj, np.float32) @ bv_a.reshape(C)).reshape(1, C)
    def p_major(w):  # [C_out-as-(s p), m] -> [p, s, m]
        return np.ascontiguousarray(
            w.reshape(6, 128, -1).transpose(1, 0, 2))
    return {
        "wq": p_major(wq_t).astype(BF16),
        "wk": p_major(wk_t).astype(BF16),
        "wv": p_major(wv_t).astype(BF16),
        "bqt": np.ascontiguousarray(bq_a.reshape(6, 128).T),
        "bkt": np.ascontiguousarray(bk_a.reshape(6, 128).T),
        "expb": np.ascontiguousarray(expb).astype(BF16),
        "wpj": p_major(wp_t).astype(BF16),
        "bpj": bp_a.astype(BF16),
    }


def _in_maps(x, shared):
    x = np.asarray(x, np.float32)
    maps = []
    for b in range(B):
        m = dict(shared)
        m["xT"] = np.ascontiguousarray(
            x[b].T.reshape(6, 128, N).transpose(1, 0, 2)).astype(BF16)
        maps.append(m)
    return maps


def kernel(**inputs):
    from concourse.bass_utils import run_bass_kernel_spmd

    nc = _build()
    shared = _prep_shared(
        inputs["shared_rel_pos"], inputs["Wqkv"], inputs["bqkv"],
        inputs["Wproj"], inputs["bproj"])
    maps = _in_maps(inputs["x"], shared)
    res = run_bass_kernel_spmd(nc, maps, core_ids=list(range(B)))
    out = np.stack([np.asarray(res.results[i]["out"], np.float32)
                    for i in range(B)])
    return out
